# revision 11
# baseline (speedup 1.0000x reference)
"""Trainium2 Bass kernel for nn_AttentionBlock (B=4, S=2048, E=512, H=8).

Sharding (no cross-core communication):
  core c owns batch b = c//2 and output rows for tokens half = c%2
  (1024 tokens).  Each core computes Q for its own 1024 tokens and K/V for
  the full batch (2048 tokens); K/V projections are duplicated between the
  two cores of a batch (cheap) so attention and the output projection are
  fully local.

Precision plan (measured rel-err ~7e-3 vs the 2e-2 gate):
  - QKV projections run as fp8e4 DoubleRow matmuls (x and W quantized to
    e4m3 on host, contraction pairs of 128-feature chunks interleaved in
    the free axis) -- 2x PE throughput.  Score error from fp8 Q/K is
    softmax-damped; V-path error is averaged by the attention weights.
  - Scores stay bf16 (K=64 contraction can't use DoubleRow -- K_part=32 DR
    matmuls don't row-pack, measured 206ns vs 124ns for packed bf16).
  - PV runs as fp8 DoubleRow over key-tile pairs: V tiles stored e4m3 with
    a ones row (sumexp), even head's exp from ACT with fp8 output, odd
    head's exp as e4m3 BITS from a DVE int8 Schraudolph tensor_scalar
    (bits = int8(A8*s + B8); the int8 tile is bitcast to fp8 for the
    matmul rhs -- writing through an fp8-tile bitcast wedges the device).
  - Out projection stays bf16 end to end (ORT/Wo fp8 measured 4.3e-2).
  - V bias and out bias are seeded into PSUM by K=1 ones-row matmuls
    (mixed bf16 seed + fp8 DR accumulation groups verified on HW), which
    frees ~16us of DVE tensor_tensor work.

Device dataflow (per core) -- same pipeline as the bf16 baseline:
  A) HAM warmup matmuls under the input DMAs, then projections (DR):
     K/Q chunk m=0, V (16 token tiles), K/Q m=1..3.  All gelus precede all
     exps so the ACT table set loads exactly 3 times.
  B) attention, q in 512-column halves, head pairs hp inside: per
     (hp,q,jp) unit two K=64 bf16 S^T matmuls per head interleaved
     even/odd (disjoint PE row groups auto-pack, 2x), exp split ACT-fp8 /
     DVE-int8, PV accumulates O^T_unnorm + sumexp via [V | 1] fp8 DR lhsT
     (M=65, key-tile pairs).  One-unit software-pipeline lag as before.
  C) normalization incrementally: reciprocal_approx_fast on sumexp rows
     (base partition 0 only), K=1 selector matmuls broadcast across 64
     partitions, DVE multiply.
  D) out = gelu(ORT chunks @ Wo.T + bo) in bf16 from PSUM (bo seeded by a
     K=1 matmul); stores alternate sync/scalar queues.
"""

import numpy as np

import concourse.bass as bass
import concourse.tile as tile
from concourse import bacc, mybir

F32 = mybir.dt.float32
BF16 = mybir.dt.bfloat16
FP8 = mybir.dt.float8e4
I8 = mybir.dt.int8
AF = mybir.ActivationFunctionType
ALU = mybir.AluOpType
PM = mybir.MatmulPerfMode

E = 512          # embed dim
H = 8            # heads
D = 64           # head dim
P = 128          # partitions
EC = E // P      # 128-row chunks of the embed dim
XC = EC // 2     # 256-row chunk PAIRS (DoubleRow contraction units)
B = 4
S = 2048
N_CORES = 8
SCALE = 0.125    # 1/sqrt(D)

# fp8 Schraudolph: e4m3 bits of exp(s/8) ~= int8(A8*s + B8).
A8 = 8.0 * SCALE * np.log2(np.e)        # 1.4427
B8 = 8.0 * (7.0 - 0.045)                # 55.64


def build(tc, io, T_KV, T_Q):
    """Emit the per-core program.  T_KV: key/value tokens; T_Q: query tokens
    (the first T_Q columns of x8_t)."""
    nc = tc.nc
    n_g = T_KV // 512        # 512-token groups for projections
    n_qg = T_Q // 512        # q groups (S^T rhs is N=512)
    n_kt = T_KV // P         # 128-token key tiles
    assert T_KV % 1024 == 0 and T_Q % 1024 == 0

    x8 = io["x8_t"]
    out = io["out"]

    with tc.tile_pool(name="persist", bufs=1) as persist, \
         tc.tile_pool(name="ps", space="PSUM", bufs=1) as psp, \
         tc.tile_pool(name="projw", bufs=1) as projw, \
         tc.tile_pool(name="xtp", bufs=2) as xtp, \
         tc.tile_pool(name="ep", bufs=4) as ep, \
         tc.tile_pool(name="tmpp", bufs=2) as tmpp, \
         tc.tile_pool(name="outp", bufs=3) as outp:
        # ---------------- persistent SBUF state ----------------
        wo_sb = [persist.tile([P, E], BF16, name=f"wo_sb{k}", tag=f"wo{k}")
                 for k in range(EC)]
        QT = [persist.tile([P, T_Q], BF16, name=f"qt_sb{m}", tag=f"qt{m}")
              for m in range(EC)]
        KT = [persist.tile([P, T_KV], BF16, name=f"kt_sb{m}", tag=f"kt{m}")
              for m in range(EC)]
        # V per key-tile PAIR: [P, head, kt-parity, 128] fp8.  Column 64 is
        # the ones row for sumexp; the DR lhsT slice [:, h] must be a fully
        # COMPACT [128, 2, 128] block -- non-compact DoubleRow operand APs
        # drop the PE to a 2x-slower path (measured 512ns vs 205ns per
        # N=512 matmul).  Cols 65..127 stay at the memset 1.0 -- they only
        # feed PSUM rows 65:127, which the drains never read.
        V6 = [persist.tile([P, H, 2, 128], FP8, name=f"v6_sb{j}", tag=f"v6{j}")
              for j in range(n_kt // 2)]
        ORT = [persist.tile([P, T_Q], BF16, name=f"ort_sb{m}", tag=f"ort{m}")
               for m in range(EC)]
        # sumexp rows live at partition 64 (even head) and 32 (odd head):
        # SBUF accesses may only start at partitions {0, 32, 64, 96} and
        # matmul operand base partitions only at {0, 32, 64}.
        SE = [persist.tile([65, T_Q], F32, name=f"se_sb{hp}", tag=f"se{hp}")
              for hp in range(H // 2)]
        G = [persist.tile([2, T_Q], F32, name=f"g_se{i}") for i in range(4)]
        g_loc = lambda h: (G[h // 2], h % 2)
        sel2 = persist.tile([65, P], F32, name="sel2")
        ones1 = persist.tile([1, P], BF16, name="ones1")
        bv_row = persist.tile([1, E], BF16, name="bv_row")
        bo_row = persist.tile([1, E], BF16, name="bo_row")

        nc.vector.memset(sel2[64:65, 0:64], 1.0)
        nc.vector.memset(sel2[32:33, 0:64], 1.0)
        nc.vector.memset(ones1, 1.0)

        # ---------------- weight / x DMA issue ----------------
        # fp8 weights; contraction row = c*256 + u*128 + p.  K/Q tiles are
        # m-outer [P, EC, 2, 128] so each lhsT slice [:, m] is compact;
        # the V tile keeps [P, 2, E] (used whole as the rhs -- compact).
        wq8 = [projw.tile([P, EC, 2, P], FP8, name=f"wq8_{c}", tag=f"wq{c}")
               for c in range(XC)]
        wk8 = [projw.tile([P, EC, 2, P], FP8, name=f"wk8_{c}", tag=f"wk{c}")
               for c in range(XC)]
        wv8 = [projw.tile([P, 2, E], FP8, name=f"wv8_{c}", tag=f"wv{c}")
               for c in range(XC)]
        bq_sb = [projw.tile([P, 1], F32, name=f"bq_sb{m}", tag=f"bq{m}")
                 for m in range(EC)]
        bk_sb = [projw.tile([P, 1], F32, name=f"bk_sb{m}", tag=f"bk{m}")
                 for m in range(EC)]
        # x fp8 tiles: gi-outer [P, 2(gi), 2(u), 512] so each K/Q rhs
        # slice [:, gi] is compact (non-compact DoubleRow operands run 2x
        # slow).  The V projection reuses these tiles with a non-compact
        # token slice -- it overlaps the DMA-bound startup window where the
        # PE would idle, so the slow path there is free and keeps the HAM
        # clock governor fed.  gp=0 rides the scalar queue, gp=1 gpsimd,
        # weights sync: three-way parallel startup.
        xt8 = {}
        for gp in range(n_g // 2):
            for c in range(XC):
                xt8[gp, c] = xtp.tile([P, 2, 2, 512], FP8,
                                      name=f"x8_g{gp}c{c}", tag=f"x8_{c}")
        for gp in range(n_g // 2):
            q = nc.scalar if gp == 0 else nc.gpsimd
            for c in range(XC):
                for u in range(2):
                    rows = slice(c * 256 + u * P, c * 256 + (u + 1) * P)
                    for gi in range(2):
                        csl = slice(gp * 1024 + gi * 512,
                                    gp * 1024 + gi * 512 + 512)
                        q.dma_start(out=xt8[gp, c][:, gi, u, :],
                                    in_=x8[rows, csl])
        for c in range(XC):
            nc.sync.dma_start(out=wk8[c],
                              in_=io["wk8m_t"][c * P:(c + 1) * P, :])
        for m in range(EC):
            nc.sync.dma_start(out=bk_sb[m],
                              in_=io["bk_col"][m * P:(m + 1) * P, :])
        for c in range(XC):
            nc.sync.dma_start(out=wq8[c],
                              in_=io["wq8m_t"][c * P:(c + 1) * P, :])
        for m in range(EC):
            nc.sync.dma_start(out=bq_sb[m],
                              in_=io["bq_col"][m * P:(m + 1) * P, :])
        for c in range(XC):
            for u in range(2):
                nc.sync.dma_start(out=wv8[c][:, u, :],
                                  in_=io["wv8_t"][c * 256 + u * P:
                                                  c * 256 + (u + 1) * P, :])
        nc.sync.dma_start(out=bv_row, in_=io["bv_rowh"])
        nc.sync.dma_start(out=bo_row, in_=io["bo_rowh"])
        for k in range(EC):
            nc.sync.dma_start(out=wo_sb[k],
                              in_=io["wo_t"][k * P:(k + 1) * P, :])
        # ones rows for sumexp (after the Pool-queue DMA issues)
        for j in range(n_kt // 2):
            nc.gpsimd.memset(V6[j], 1.0)

        # ---------------- emission helpers ----------------
        def k_chunk(m, gp):
            """K^T chunk for feature rows m*128:(m+1)*128, token group pair
            gp, fp8 DoubleRow."""
            ps = psp.tile([P, 1024], F32, name=f"ps_k{gp}_{m}",
                          tag="st", bufs=3)
            for gi in range(2):
                for c in range(XC):
                    nc.tensor.matmul(
                        ps[:, gi * 512:(gi + 1) * 512],
                        lhsT=wk8[c][:, m, :, :],
                        rhs=xt8[gp, c][:, gi, :, :],
                        start=(c == 0), stop=(c == XC - 1),
                        perf_mode=PM.DoubleRow)
            nc.scalar.activation(KT[m][:, gp * 1024:(gp + 1) * 1024],
                                 ps, AF.Gelu, bias=bk_sb[m])

        def q_chunk(m):
            ps = psp.tile([P, 1024], F32, name=f"ps_q_{m}", tag="st", bufs=3)
            for gi in range(2):
                for c in range(XC):
                    nc.tensor.matmul(
                        ps[:, gi * 512:(gi + 1) * 512],
                        lhsT=wq8[c][:, m, :, :],
                        rhs=xt8[0, c][:, gi, :, :],
                        start=(c == 0), stop=(c == XC - 1),
                        perf_mode=PM.DoubleRow)
            nc.scalar.activation(QT[m], ps, AF.Gelu, bias=bq_sb[m])

        def kq_chunk(m):
            k_chunk(m, 0)
            k_chunk(m, 1)
            q_chunk(m)

        def v_tiles(trange):
            """V natural [token, feature], fp8 DR; bias row seeded into
            PSUM by a K=1 bf16 ones matmul."""
            for t in trange:
                gp, s8 = divmod(t, 8)
                ps = psp.tile([P, E], F32, name=f"ps_v{t}", tag="st", bufs=3)
                nc.tensor.matmul(ps, lhsT=ones1, rhs=bv_row,
                                 start=True, stop=False)
                gi, s4 = divmod(s8, 4)
                for c in range(XC):
                    nc.tensor.matmul(ps,
                                     lhsT=xt8[gp, c][:, gi, :,
                                                     s4 * P:(s4 + 1) * P],
                                     rhs=wv8[c],
                                     start=False, stop=(c == XC - 1),
                                     perf_mode=PM.DoubleRow)
                vst = xtp.tile([P, E], FP8, name=f"vst{t}", tag="vst",
                               bufs=3)
                nc.scalar.activation(vst, ps, AF.Gelu)
                j, up = divmod(t, 2)
                nc.sync.dma_start(
                    out=V6[j][:, :, up, 0:64],
                    in_=vst.rearrange("p (h d) -> p h d", h=H))

        def st_exp_unit(hp, q, jp):
            """S^T matmuls + exp for one (head pair, q-half, key-tile pair);
            returns the e tiles for the deferred PV emission."""
            qsl = slice(q * 512, (q + 1) * 512)
            st0 = psp.tile([P, 1024], F32, name=f"st0_{hp}{q}{jp}",
                           tag="st", bufs=3)
            st1 = psp.tile([P, 1024], F32, name=f"st1_{hp}{q}{jp}",
                           tag="st", bufs=3)
            # interleave even/odd so adjacent matmuls use disjoint PE
            # row groups (0:64 vs 64:128) and run concurrently.
            for u in range(2):
                kt = jp * 2 + u
                ksl = slice(kt * P, (kt + 1) * P)
                usl = slice(u * 512, (u + 1) * 512)
                nc.tensor.matmul(st0[:, usl], lhsT=KT[hp][0:64, ksl],
                                 rhs=QT[hp][0:64, qsl],
                                 start=True, stop=True)
                nc.tensor.matmul(st1[:, usl], lhsT=KT[hp][64:128, ksl],
                                 rhs=QT[hp][64:128, qsl],
                                 start=True, stop=True)
            e0 = ep.tile([P, 1024], FP8, name=f"e0_{hp}{q}{jp}", tag="e0",
                         bufs=6)
            # even head: exact exp on ScalarE with fp8 output; odd head:
            # Schraudolph e4m3-bits on VectorE into a native int8 tile --
            # except every 7th unit, where ScalarE takes it too (exact exp,
            # fp8 tile) to balance engine load (ACT ~1.11us vs DVE ~1.45us
            # per unit otherwise).
            nc.scalar.activation(e0, st0, AF.Exp, scale=SCALE)
            self_cnt = unit_counter[0]
            unit_counter[0] += 1
            if self_cnt % 7 == 6:
                e1 = ep.tile([P, 1024], FP8, name=f"e1a_{hp}{q}{jp}",
                             tag="e1a", bufs=4)
                nc.scalar.activation(e1, st1, AF.Exp, scale=SCALE)
            else:
                e1 = ep.tile([P, 1024], I8, name=f"e1_{hp}{q}{jp}",
                             tag="e1", bufs=6)
                nc.vector.tensor_scalar(out=e1, in0=st1,
                                        scalar1=float(A8), scalar2=float(B8),
                                        op0=ALU.mult, op1=ALU.add)
            return e0, e1

        def pv_unit(hp, q, jp, e0, e1):
            pv0, pv1 = pv[hp, q, 0], pv[hp, q, 1]
            he, ho = 2 * hp, 2 * hp + 1
            ev0 = e0.rearrange("p (two n) -> p two n", two=2)
            e1f = e1 if e1.dtype == FP8 else e1.bitcast(FP8)
            ev1 = e1f.rearrange("p (two n) -> p two n", two=2)
            first, last = jp == 0, jp == n_kt // 2 - 1
            nc.tensor.matmul(pv0, lhsT=V6[jp][:, he], rhs=ev0,
                             start=first, stop=last, perf_mode=PM.DoubleRow)
            nc.tensor.matmul(pv1, lhsT=V6[jp][:, ho], rhs=ev1,
                             start=first, stop=last, perf_mode=PM.DoubleRow)

        def drain(hp, q, tail=False):
            """Move O^T_unnorm into ORT and sumexp rows into G.  Even head
            rows are partition-aligned; the odd head hops across partitions
            via SBUF + DMA.  The two big copies ride ScalarE (it has more
            per-unit slack than VectorE).  tail=True short-circuits the G
            gather: the reciprocal runs right here on the staging tiles and
            lands straight in SE, shortening the end-of-kernel norm chain."""
            pv0, pv1 = pv[hp, q, 0], pv[hp, q, 1]
            he, ho = 2 * hp, 2 * hp + 1
            qsl = slice(q * 512, (q + 1) * 512)
            cp = nc.scalar.copy if tail else \
                (lambda o, i: nc.vector.tensor_copy(o, i))
            cp(ORT[hp][0:64, qsl], pv0[0:64, :])
            tmp_v = tmpp.tile([64, 512], BF16, name=f"tv_{hp}_{q}", tag="tv")
            ts0 = tmpp.tile([65, 512], F32, name=f"ts0_{hp}_{q}", tag="ts0")
            ts1 = tmpp.tile([65, 512], F32, name=f"ts1_{hp}_{q}", tag="ts1")
            cp(tmp_v, pv1[0:64, :])
            nc.vector.tensor_copy(ts0[64:65, :], pv0[64:65, :])
            nc.vector.tensor_copy(ts1[64:65, :], pv1[64:65, :])
            nc.sync.dma_start(out=ORT[hp][64:128, qsl], in_=tmp_v)
            ge, re = g_loc(he)
            go, ro = g_loc(ho)
            nc.sync.dma_start(out=ge[re:re + 1, qsl], in_=ts0[64:65, :])
            nc.sync.dma_start(out=go[ro:ro + 1, qsl], in_=ts1[64:65, :])
            if tail:
                # reciprocal immediately (the custom DVE op only works at
                # base partition 0, hence via G) and straight into SE so the
                # end-of-kernel norm chain is short.
                gq = G[hp]
                nc.vector.reciprocal_approx_fast(out=gq[0:2, qsl],
                                                 in_=gq[0:2, qsl])
                nc.sync.dma_start(out=SE[hp][64:65, qsl],
                                  in_=gq[0:1, qsl])
                nc.sync.dma_start(out=SE[hp][32:33, qsl],
                                  in_=gq[1:2, qsl])

        def norm(hps, q, tail=False):
            """Normalize ORT[:, q-half] for the given head pairs (hps must
            be [0,1], [2,3] or [0,1,2,3] so the reciprocal APs start at
            partition 0): reciprocal over the G rows, partition broadcast
            via K=1 selector matmuls, DVE multiply.  tail=True means the
            drains already reciprocated into SE directly."""
            qsl = slice(q * 512, (q + 1) * 512)
            if not tail:
                for hp in hps:
                    nc.vector.reciprocal_approx_fast(out=G[hp][0:2, qsl],
                                                     in_=G[hp][0:2, qsl])
                for hp in hps:
                    nc.sync.dma_start(out=SE[hp][64:65, qsl],
                                      in_=G[hp][0:1, qsl])
                    nc.sync.dma_start(out=SE[hp][32:33, qsl],
                                      in_=G[hp][1:2, qsl])
            for hp in hps:
                R = psp.tile([P, 512], F32, name=f"R_{hp}_{q}", tag="st",
                             bufs=3)
                nc.tensor.matmul(R[0:64, :], lhsT=sel2[64:65, 0:64],
                                 rhs=SE[hp][64:65, qsl],
                                 start=True, stop=True)
                nc.tensor.matmul(R[64:128, :], lhsT=sel2[32:33, 0:64],
                                 rhs=SE[hp][32:33, qsl],
                                 start=True, stop=True)
                nc.vector.tensor_mul(ORT[hp][:, qsl], ORT[hp][:, qsl], R)

        def out_proj(ts):
            for t in ts:
                tsl = slice(t * P, (t + 1) * P)
                ps = psp.tile([P, E], F32, name=f"ps_o{t}", tag="st", bufs=3)
                nc.tensor.matmul(ps, lhsT=ones1, rhs=bo_row,
                                 start=True, stop=False)
                for m in range(EC):
                    nc.tensor.matmul(ps, lhsT=ORT[m][:, tsl], rhs=wo_sb[m],
                                     start=False, stop=(m == EC - 1))
                ot = outp.tile([P, E], F32, name=f"ot_{t}", tag="ot")
                nc.scalar.activation(ot, ps, AF.Gelu)
                # alternate queues so the eight 256KB stores don't serialize
                eng = nc.sync if t % 2 == 0 else nc.scalar
                eng.dma_start(out=out[tsl, :], in_=ot)

        # ---------------- program ----------------
        unit_counter = [0]
        pv = {}
        for q in range(n_qg):
            for hp in range(H // 2):
                pv[hp, q, 0] = psp.tile([P, 512], F32, name=f"pv0_{hp}_{q}",
                                        tag="pv", bufs=2)
                pv[hp, q, 1] = psp.tile([P, 512], F32, name=f"pv1_{hp}_{q}",
                                        tag="pv", bufs=2)

        # HAM warmup: ~3.5us of junk matmuls while the input DMAs land so
        # the PE clock is at 2.4 GHz when the first projection runs.
        warm = persist.tile([64, 256], BF16, name="warm")
        nc.vector.memset(warm, 0.5)
        wps = psp.tile([64, 512], F32, name="warm_ps", tag="st", bufs=3)
        for _ in range(96):
            nc.tensor.matmul(wps[:, 0:128], lhsT=warm[:, 0:64],
                             rhs=warm[:, 0:128], start=True, stop=True)

        k_chunk(0, 0)
        k_chunk(0, 1)
        k_chunk(1, 0)
        k_chunk(1, 1)
        q_chunk(0)
        q_chunk(1)
        v_tiles(range(0, 16))
        k_chunk(2, 0)
        k_chunk(2, 1)
        q_chunk(2)
        k_chunk(3, 0)
        k_chunk(3, 1)
        q_chunk(3)

        # Attention stream, software-pipelined with a TWO-unit lag: each
        # unit's PV matmuls are emitted after the st matmuls of the unit
        # after next, so the PE never waits on the ~2us exp latency (with
        # only 6 matmuls per unit a one-unit lag stalls ~1us per unit).
        from collections import deque
        pends = deque()

        def push(fn):
            pends.append(fn)
            if len(pends) > 2:
                pends.popleft()()

        def run_block(hp, q, tail=False):
            for jp in range(n_kt // 2):
                e0, e1 = st_exp_unit(hp, q, jp)
                # junk weight loads: free PE-queue activity that fills
                # dependency stalls so the HAM clock governor keeps the
                # PE at 2.4 GHz (sparse windows re-throttle to 1.2 GHz).
                nc.tensor.ldweights(warm[:, 0:64])
                nc.tensor.ldweights(warm[:, 64:128])
                last = jp == n_kt // 2 - 1
                push(lambda hp=hp, q=q, jp=jp, e0=e0, e1=e1, last=last,
                     tail=tail:
                     (pv_unit(hp, q, jp, e0, e1),
                      drain(hp, q, tail=tail) if last else None))

        def flush():
            while pends:
                pends.popleft()()

        run_block(0, 0)
        run_block(1, 0)
        run_block(2, 0)
        run_block(3, 0)
        run_block(0, 1)
        norm([0, 1], 0)
        run_block(1, 1)
        norm([2, 3], 0)
        run_block(2, 1, tail=True)
        norm([0, 1], 1)
        run_block(3, 1, tail=True)
        flush()
        # out tiles 0..3 only need the long-finished q0 normalization --
        # emit them ahead of the tail norm so the PE works through them
        # while the (2,3)/q1 reciprocal chain resolves.
        out_proj(range(0, 4))
        norm([2, 3], 1, tail=True)
        out_proj(range(4, 8))


def make_nc(T_KV, T_Q, num_devices=N_CORES, debug=False):
    nc = bacc.Bacc("TRN2", target_bir_lowering=False, debug=debug,
                   num_devices=num_devices)
    io = {
        "x8_t": nc.dram_tensor("x8_t", [E, T_KV], FP8,
                               kind="ExternalInput").ap(),
        "wq8m_t": nc.dram_tensor("wq8m_t", [2 * P, 4 * 256], FP8,
                                 kind="ExternalInput").ap(),
        "wk8m_t": nc.dram_tensor("wk8m_t", [2 * P, 4 * 256], FP8,
                                 kind="ExternalInput").ap(),
        "wv8_t": nc.dram_tensor("wv8_t", [E, E], FP8,
                                kind="ExternalInput").ap(),
        "wo_t": nc.dram_tensor("wo_t", [E, E], BF16,
                               kind="ExternalInput").ap(),
        "bq_col": nc.dram_tensor("bq_col", [E, 1], F32,
                                 kind="ExternalInput").ap(),
        "bk_col": nc.dram_tensor("bk_col", [E, 1], F32,
                                 kind="ExternalInput").ap(),
        "bv_rowh": nc.dram_tensor("bv_rowh", [1, E], BF16,
                                  kind="ExternalInput").ap(),
        "bo_rowh": nc.dram_tensor("bo_rowh", [1, E], BF16,
                                  kind="ExternalInput").ap(),

        "out": nc.dram_tensor("out", [T_Q, E], F32, kind="ExternalOutput").ap(),
    }
    with tile.TileContext(nc) as tc:
        build(tc, io, T_KV=T_KV, T_Q=T_Q)
    nc.compile()
    return nc


def make_in_maps(x, Wq, bq, Wk, bk, Wv, bv, Wo, bo):
    import ml_dtypes
    f8 = ml_dtypes.float8_e4m3fn
    bf = ml_dtypes.bfloat16
    cast8 = lambda a: np.ascontiguousarray(np.asarray(a).astype(f8))
    castb = lambda a: np.ascontiguousarray(np.asarray(a).astype(bf))
    castf = lambda a: np.ascontiguousarray(np.asarray(a, dtype=np.float32))
    def w_mlayout(Wt):
        # [E, E] -> [256, 2048]: row c*128+p, col m*256+u*128+r holds
        # Wt[c*256+u*128+p, m*128+r]
        a = Wt.reshape(2, 2, P, 4, P)
        return np.ascontiguousarray(
            a.transpose(0, 2, 3, 1, 4).reshape(2 * P, -1))

    base = {
        "wq8m_t": w_mlayout(cast8(np.asarray(Wq).T)),
        "wk8m_t": w_mlayout(cast8(np.asarray(Wk).T)),
        "wv8_t": cast8(np.asarray(Wv).T),
        "wo_t": castb(np.asarray(Wo).T),
        "bq_col": castf(np.asarray(bq)[:, None]),
        "bk_col": castf(np.asarray(bk)[:, None]),
        "bv_rowh": castb(np.asarray(bv)[None, :]),
        "bo_rowh": castb(np.asarray(bo)[None, :]),
    }
    x = np.asarray(x)
    half_len = S // 2
    in_maps = []
    for c in range(N_CORES):
        b, half = divmod(c, 2)
        xb = x[b]
        mine = xb[half * half_len:(half + 1) * half_len]
        oth = xb[(1 - half) * half_len:(2 - half) * half_len]
        m = dict(base)
        m["x8_t"] = cast8(np.concatenate([mine, oth], axis=0).T)
        in_maps.append(m)
    return in_maps


_NC_CACHE = {}


def _get_full_nc():
    if "full" not in _NC_CACHE:
        _NC_CACHE["full"] = make_nc(T_KV=S, T_Q=S // 2)
    return _NC_CACHE["full"]


def run_on_hw(in_maps, trace=False, **kw):
    from concourse.bass_utils import run_bass_kernel_spmd
    nc = _get_full_nc()
    return run_bass_kernel_spmd(nc, in_maps, core_ids=list(range(N_CORES)),
                                trace=trace, **kw)


def kernel(x, Wq, bq, Wk, bk, Wv, bv, Wo, bo):
    in_maps = make_in_maps(x, Wq, bq, Wk, bk, Wv, bv, Wo, bo)
    res = run_on_hw(in_maps)
    half_len = S // 2
    out = np.empty((B, S, E), np.float32)
    for c in range(N_CORES):
        b, half = divmod(c, 2)
        out[b, half * half_len:(half + 1) * half_len, :] = \
            res.results[c]["out"]
    return out
